# revision 1
# baseline (speedup 1.0000x reference)
"""Causal self-attention (B=4, T=2048, C=1024, 16 heads, fp32) on 8 TRN2 NeuronCores.

Sharding: 8 cores = 4 batches x 2 head-groups (8 heads each).  Each core runs an
identical program on its (batch, head-group) shard:

  phase 1: QKV projection.  x^T (pre-transposed on host) enters SBUF; Q^T and K^T
           are produced in [head*64+d, T] layout (matmul lhsT = Wqkv column slab),
           V in natural [T, head*64+d] layout (lhsT = x^T tile), augmented with a
           ones column per head so that the attention AV matmul also produces the
           softmax denominator for free.
  phase 2: flash-style causal attention per head pair, all matmuls in fp32r
           (TensorE full rate, ~1.5e-4 rms rounding).  S^T[k,q] blocks via
           K=64 matmuls packed two heads per PE pass (row groups 0/64) into one
           2-bank PSUM tile, one 1024-wide exp on ScalarE straight out of PSUM,
           block-causal masking via a single triangular 128x128 fp32r mask
           (tensor_mul) plus zero-fill, with diagonal S/AV matmuls narrowed to
           the un-masked q-range (>=256 to stay at fp32r full rate).  O^T and
           the softmax denominators accumulate together in PSUM over key
           blocks (ones-augmented V).  Normalization happens on the PSUM->SBUF
           copy-out (reciprocal + partition_broadcast + tensor_mul), writing
           y^T over the dead Q^T chunk (tile aliasing, no extra SBUF).
  phase 3: output projection from y^T layout (lhsT = y^T slice), partial
           [T, C] result per core, DMA'd out.  Emission of phase-1 chunks,
           attention q-chunks, and projection is interleaved so every engine's
           program-order queue (and pool slot cycling) pipelines across phases.

Host side: per-batch pairs of partial outputs are summed (the 2-way "all-reduce"
of the row-sharded Wproj), plus the rank-1 bias correction
(bqkv_v @ Wproj + bproj) which commutes with attention because softmax rows sum
to one.  Softmax max-subtraction is skipped: scores are ~N(0,1) after the 1/8
scale, exp never overflows, and the result is mathematically identical.
"""
import numpy as np

import concourse.bass as bass  # noqa: F401  (bass must be imported before tile)
import concourse.tile as tile
from concourse import mybir
from concourse.bacc import Bacc
from concourse.bass_utils import run_bass_kernel_spmd

F32 = mybir.dt.float32
F32R = mybir.dt.float32r

B, T, C = 4, 2048, 1024
NH = 16          # total heads
D = 64           # head dim
G = 2            # head groups (cores per batch)
HPG = NH // G    # heads per group = 8
GC = HPG * D     # columns per group = 512
CT = C // 128    # contraction tiles = 8
QCW = 512        # q-chunk width
NQC = T // QCW   # 4 q-chunks
NTT = T // 128   # 16 t-tiles
NHP = HPG // 2   # head pairs per core = 4
EXP = mybir.ActivationFunctionType.Exp
CPY = mybir.ActivationFunctionType.Copy


def build():
    nc = Bacc()
    xT = nc.dram_tensor("xT", [C, T], F32, kind="ExternalInput")
    wqk = nc.dram_tensor("wqk", [C, 2 * GC], F32, kind="ExternalInput")
    wv = nc.dram_tensor("wv", [C, GC], F32, kind="ExternalInput")
    wp = nc.dram_tensor("wp", [GC, C], F32, kind="ExternalInput")
    bqk = nc.dram_tensor("bqk", [128, 2 * GC // 128], F32, kind="ExternalInput")
    out = nc.dram_tensor("out", [T, C], F32, kind="ExternalOutput")

    PCW = 256            # phase-1 x chunk width
    NPC = T // PCW       # 8 phase-1 chunks

    with tile.TileContext(nc) as tc:
        with (
            tc.tile_pool(name="persist", bufs=1) as pp,
            tc.tile_pool(name="stg", bufs=2) as stg,
            tc.tile_pool(name="w1", bufs=1) as w1p,
            tc.tile_pool(name="xc", bufs=2) as xcp,
            tc.tile_pool(name="pt", bufs=3) as ptp,
            tc.tile_pool(name="rb", bufs=2) as rbp,
            tc.tile_pool(name="ost", bufs=2) as ost,
            tc.tile_pool(name="ps", bufs=2, space="PSUM") as ps,
            tc.tile_pool(name="psS", bufs=2, space="PSUM") as psS,
            tc.tile_pool(name="psO", bufs=1, space="PSUM") as psO,
        ):
            # long-lived SBUF tensors.  QT[j][qc] doubles as y^T storage: the
            # normalized O^T for (hp, qc) overwrites the Q^T chunk it consumed.
            QT = [[pp.tile([128, QCW], F32R, tag=f"qt{j}_{q}", name=f"qt{j}_{q}")
                   for q in range(NQC)] for j in range(NHP)]
            KT = [[pp.tile([128, QCW], F32R, tag=f"kt{j}_{q}", name=f"kt{j}_{q}")
                   for q in range(NQC)] for j in range(NHP)]
            YT = QT
            VA = [pp.tile([128, HPG, D + 1], F32R, tag=f"va{t}", name=f"va{t}") for t in range(NTT)]
            WP = [pp.tile([128, C], F32R, tag=f"wpr{j}", name=f"wpr{j}") for j in range(GC // 128)]
            bqk_sb = pp.tile([128, 2 * GC // 128], F32)
            nc.sync.dma_start(out=bqk_sb, in_=bqk[:])
            ones32 = pp.tile([128, HPG, 1], F32)
            nc.vector.memset(ones32, 1.0)
            # upper-triangular (keep k<=q) mask for diagonal 128x128 sub-blocks
            tri32 = pp.tile([128, 128], F32)
            nc.vector.memset(tri32, 1.0)
            nc.gpsimd.affine_select(
                out=tri32, in_=tri32, pattern=[[1, 128]],
                compare_op=mybir.AluOpType.is_ge, fill=0.0,
                base=0, channel_multiplier=-1,
            )
            tri = pp.tile([128, 128], F32R)
            nc.vector.tensor_copy(tri, tri32)

            # weights: DMA + cast to fp32r
            WQK, WV = [], []
            for c in range(CT):
                wr = w1p.tile([128, 2 * GC], F32R, tag=f"wqk{c}", name=f"wqk{c}")
                for h in range(2):
                    s = stg.tile([128, GC], F32, tag="stgs", name="stgs")
                    nc.sync.dma_start(out=s, in_=wqk[128 * c:128 * (c + 1), GC * h:GC * (h + 1)])
                    nc.vector.tensor_copy(wr[:, GC * h:GC * (h + 1)], s)
                WQK.append(wr)

            def load_wv():
                for c in range(CT):
                    s = stg.tile([128, GC], F32, tag="stgs", name="stgs")
                    nc.sync.dma_start(out=s, in_=wv[128 * c:128 * (c + 1), :])
                    vr = w1p.tile([128, GC], F32R, tag=f"wv{c}", name=f"wv{c}")
                    nc.vector.tensor_copy(vr, s)
                    WV.append(vr)

            def load_wp():
                for j in range(GC // 128):
                    for h in range(2):
                        s = stg.tile([128, GC], F32, tag="stgs", name="stgs")
                        nc.sync.dma_start(out=s, in_=wp[128 * j:128 * (j + 1), GC * h:GC * (h + 1)])
                        nc.vector.tensor_copy(WP[j][:, GC * h:GC * (h + 1)], s)

            XC_by_ch = {}

            def phase1_chunk(ch, skip_v=False):
                XC = []
                for c in range(CT):
                    s = stg.tile([128, PCW], F32, tag="stgx", name="stgx")
                    nc.sync.dma_start(
                        out=s, in_=xT[128 * c:128 * (c + 1), PCW * ch:PCW * (ch + 1)]
                    )
                    xr = xcp.tile([128, PCW], F32R, tag=f"xc{c}", name=f"xc{c}")
                    nc.vector.tensor_copy(xr, s)
                    XC.append(xr)
                XC_by_ch[ch] = XC
                # Q^T / K^T row-tiles (m<4 -> Q pair-tile m, m>=4 -> K pair-tile m-4)
                for m in range(2 * GC // 128):
                    acc = ps.tile([128, 512], F32, tag="pp", name="pp")[:, :PCW]
                    for c in range(CT):
                        nc.tensor.matmul(
                            acc, WQK[c][:, 128 * m:128 * (m + 1)], XC[c],
                            start=(c == 0), stop=(c == CT - 1),
                        )
                    dst = QT[m][ch // 2] if m < NHP else KT[m - NHP][ch // 2]
                    off = PCW * (ch % 2)
                    nc.vector.tensor_scalar_add(
                        dst[:, off:off + PCW], acc, bqk_sb[:, m:m + 1]
                    )
                if not skip_v:
                    phase1_v(ch)

            def phase1_v(ch):
                XC = XC_by_ch[ch]
                for ti in range(PCW // 128):
                    t = (PCW // 128) * ch + ti
                    acc = ps.tile([128, 512], F32, tag="pp", name="pp")
                    for c in range(CT):
                        nc.tensor.matmul(
                            acc, XC[c][:, 128 * ti:128 * (ti + 1)], WV[c],
                            start=(c == 0), stop=(c == CT - 1),
                        )
                    nc.vector.tensor_copy(
                        VA[t][:, :, 0:D], acc.rearrange("p (h d) -> p h d", h=HPG)
                    )
                    nc.vector.tensor_copy(VA[t][:, :, D:D + 1], ones32)

            def attention_qc(qc):
                kbmax = 4 * (qc + 1)
                for hp in range(NHP):
                    O = psO.tile([D + 1, 2 * QCW], F32, tag="o", name="o")
                    for kb in range(kbmax):
                        j = kb - 4 * qc
                        # q-columns < 128*j are fully causal-masked for this
                        # k-block: narrow S/AV to q >= s_off (fp32r needs
                        # moving free >= 256, so cap s_off at QCW-256)
                        s_off = min(128 * j, QCW - 256) if j > 0 else 0
                        S = psS.tile([128, 2 * QCW], F32, tag="s", name="s")
                        for ph in range(2):
                            p_sl = slice(64 * ph, 64 * (ph + 1))
                            nc.tensor.matmul(
                                S[:, QCW * ph + s_off:QCW * (ph + 1)],
                                KT[hp][kb // 4][p_sl, 128 * (kb % 4):128 * (kb % 4 + 1)],
                                QT[hp][qc][p_sl, s_off:],
                                start=True, stop=True,
                            )
                        P = ptp.tile([128, 2 * QCW], F32R, tag="p", name="p")
                        nc.scalar.activation(out=P, in_=S, func=EXP, scale=0.125)
                        if j >= 0:
                            for ph in range(2):
                                off = QCW * ph + 128 * j
                                nc.vector.tensor_mul(
                                    P[:, off:off + 128], P[:, off:off + 128], tri
                                )
                                if 128 * j > s_off:
                                    nc.vector.tensor_scalar_mul(
                                        P[:, QCW * ph + s_off:QCW * ph + 128 * j],
                                        P[:, QCW * ph + s_off:QCW * ph + 128 * j], 0.0,
                                    )
                        for ph in range(2):
                            nc.tensor.matmul(
                                O[:, QCW * ph + s_off:QCW * (ph + 1)],
                                VA[kb][:, 2 * hp + ph, :],
                                P[:, QCW * ph + s_off:QCW * (ph + 1)],
                                start=(kb == 0), stop=(kb == kbmax - 1),
                            )
                    rc = rbp.tile([1, 2 * QCW], F32, tag="rc", name="rc", bufs=1)
                    nc.vector.reciprocal(rc, O[D:D + 1, :])
                    for ph in range(2):
                        rb = rbp.tile([64, QCW], F32, tag="rb", name="rb")
                        nc.gpsimd.partition_broadcast(rb, rc[0:1, QCW * ph:QCW * (ph + 1)])
                        nc.vector.tensor_mul(
                            YT[hp][qc][64 * ph:64 * (ph + 1), :],
                            O[0:D, QCW * ph:QCW * (ph + 1)], rb,
                        )

            def proj_qc(qc):
                for ti in range(4):
                    t = 4 * qc + ti
                    for nn in range(2):
                        acc = ps.tile([128, 512], F32, tag="pp", name="pp")
                        for j in range(GC // 128):
                            nc.tensor.matmul(
                                acc,
                                YT[j][qc][:, 128 * ti:128 * (ti + 1)],
                                WP[j][:, 512 * nn:512 * (nn + 1)],
                                start=(j == 0), stop=(j == GC // 128 - 1),
                            )
                        o = ost.tile([128, 512], F32, tag="o", name="o")
                        nc.vector.tensor_copy(o, acc)
                        nc.sync.dma_start(
                            out=out[128 * t:128 * (t + 1), 512 * nn:512 * (nn + 1)],
                            in_=o,
                        )

            # interleaved emission: attention for q-chunk qc right after the
            # phase-1 chunks that produce its inputs; proj delayed one pair so
            # its pool-slot allocations never gate the next phase-1 chunk
            for ch in range(NPC):
                if ch == 1:
                    load_wv()
                    phase1_v(0)
                phase1_chunk(ch, skip_v=(ch == 0))
                if ch % 2 == 1:
                    qc = ch // 2
                    if qc == 0:
                        load_wp()
                    else:
                        proj_qc(qc - 1)
                    attention_qc(qc)
            proj_qc(NQC - 1)
    nc.finalize()
    return nc


_NC = None


def _get_nc():
    global _NC
    if _NC is None:
        _NC = build()
    return _NC


def _shard(x, Wqkv, bqkv, Wproj):
    in_maps = []
    for core in range(8):
        b, g = core // G, core % G
        cs = slice(GC * g, GC * (g + 1))
        wqk_h = np.concatenate([Wqkv[:, cs], Wqkv[:, C:][:, cs]], axis=1)
        bqk_h = np.concatenate([bqkv[cs], bqkv[C:][cs.start:cs.stop]])
        in_maps.append({
            "xT": np.ascontiguousarray(x[b].T),
            "wqk": np.ascontiguousarray(wqk_h),
            "wv": np.ascontiguousarray(Wqkv[:, 2 * C:][:, cs]),
            "wp": np.ascontiguousarray(Wproj[cs, :]),
            "bqk": np.ascontiguousarray(bqk_h.reshape(2 * GC // 128, 128).T),
        })
    return in_maps


def kernel(x, Wqkv, bqkv, Wproj, bproj, _want_results=False, **run_kwargs):
    x = np.ascontiguousarray(np.asarray(x, dtype=np.float32))
    Wqkv = np.ascontiguousarray(np.asarray(Wqkv, dtype=np.float32))
    bqkv = np.ascontiguousarray(np.asarray(bqkv, dtype=np.float32))
    Wproj = np.ascontiguousarray(np.asarray(Wproj, dtype=np.float32))
    bproj = np.ascontiguousarray(np.asarray(bproj, dtype=np.float32))

    nc = _get_nc()
    in_maps = _shard(x, Wqkv, bqkv, Wproj)
    res = run_bass_kernel_spmd(nc, in_maps, core_ids=list(range(8)), **run_kwargs)

    out = np.empty((B, T, C), dtype=np.float32)
    for b in range(B):
        out[b] = res.results[G * b]["out"]
        for g in range(1, G):
            out[b] += res.results[G * b + g]["out"]
    # rank-1 corrections: v-bias (rows of softmax sum to 1) and proj bias
    out += bqkv[2 * C:] @ Wproj + bproj
    if _want_results:
        return out, res
    return out

